# revision 2
# baseline (speedup 1.0000x reference)
"""Trainium2 Bass kernel v2 for GQA attention (B=2, S=2048, D=2048, H=16, HK=4).

Sharding: 8 devices = batch(2) x kv-groups(4), as baseline: each device owns
one batch element and one GQA group (4 q-heads + 1 kv-head); wq/wk/wv
column-parallel, wo row-parallel (host sums the 4 partials per batch element).

v2 changes vs baseline:
  - all matmuls bf16 (1 cycle/row at ANY moving width; fp32r needed >=256).
    Numerics verified host-side: all-bf16 absmax-rel err 3.1e-3 << 2e-2 gate.
  - denominator row-sum matmuls (dd, ~72K PE rows) and one-hot broadcast
    matmuls (bb, 8K rows) eliminated: AV is computed in [sq, dk] orientation
    (exp tile stationary, v moving) with a ones-column appended to v, so the
    softmax denominator rides along as output column 128. Normalization is
    then a per-partition scalar multiply on ACT (native broadcast).
  - causal diagonal handled by a multiplicative 0/1 triangle on the exp tile
    (DVE, bf16 2x) instead of additive -1e9 + padded fp32r tiles; score
    tiles use exact widths {512,384,256,128}.
  - oh tiles transposed back to [dk, sq] on PE (bf16, 64x128 rows) for the
    row-parallel wo matmul.
  - QKV projection (chunk c+1) and O-proj (ready sq-tiles) are interleaved
    into the attention t-loop so the ACT exp stream (~100us) hides under
    projection PE work instead of stalling the attention phase.

PSUM budget: pp(2: QKV out + transposes + O-proj out) + ss(3: scores) +
oq(3: 8 attention accumulators packed 3x129 cols per bank) = 8 banks.
"""

import math

import numpy as np
import ml_dtypes

import concourse.bacc as bacc
import concourse.tile as tile
from concourse import mybir
from concourse.bass_utils import run_bass_kernel_spmd

B, S, D = 2, 2048, 2048
H, HK, DK = 16, 4, 128
REP = H // HK
NDEV = 8
P = 128
CH = 512
ND = D // P          # 16 k-step tiles per matmul
NT = S // P          # 16 sk/sq tiles
F32 = mybir.dt.float32
BF16 = mybir.dt.bfloat16
NPBF = ml_dtypes.bfloat16


def _build(s_len=S, reps=1):
    nch = s_len // CH
    nt = s_len // P
    scale = 1.0 / math.sqrt(DK)
    Exp = mybir.ActivationFunctionType.Exp

    nc = bacc.Bacc("TRN2", target_bir_lowering=False, debug=False,
                   enable_asserts=False, num_devices=1)
    xT = nc.dram_tensor("xT", [D, s_len], BF16, kind="ExternalInput").ap()
    W = nc.dram_tensor("W", [P, 6 * D], BF16, kind="ExternalInput").ap()
    woT = nc.dram_tensor("woT", [REP * DK, D], BF16, kind="ExternalInput").ap()
    CSt = nc.dram_tensor("CS", [P, s_len], F32, kind="ExternalInput").ap()
    SCt = nc.dram_tensor("SC", [P, s_len], F32, kind="ExternalInput").ap()
    T01 = nc.dram_tensor("T01", [P, P], BF16, kind="ExternalInput").ap()
    IDB = nc.dram_tensor("IDB", [P, P], BF16, kind="ExternalInput").ap()
    ONESt = nc.dram_tensor("ONES", [P, nt], BF16, kind="ExternalInput").ap()
    BIAS = nc.dram_tensor("BIAS", [P, 6], F32, kind="ExternalInput").ap()
    BIAS2 = nc.dram_tensor("BIAS2", [P, 6], F32, kind="ExternalInput").ap()
    out = nc.dram_tensor("out", [s_len, D], F32, kind="ExternalOutput").ap()

    with tile.TileContext(nc) as tc:
      for _rep in range(reps):
        with tc.tile_pool(name="consts", bufs=1) as consts, \
             tc.tile_pool(name="qk", bufs=1) as qkpool, \
             tc.tile_pool(name="vv", bufs=1) as vvpool, \
             tc.tile_pool(name="oh", bufs=1) as ohpool, \
             tc.tile_pool(name="wst", bufs=1) as wpool, \
             tc.tile_pool(name="wo", bufs=1) as wopool, \
             tc.tile_pool(name="xh", bufs=2) as xpool, \
             tc.tile_pool(name="rope", bufs=3) as rpool, \
             tc.tile_pool(name="ew", bufs=2) as epool, \
             tc.tile_pool(name="oht", bufs=2) as otpool, \
             tc.tile_pool(name="fo", bufs=3) as fopool, \
             tc.tile_pool(name="den", bufs=2) as dpool, \
             tc.tile_pool(name="pp", bufs=2, space="PSUM") as pps, \
             tc.tile_pool(name="ss", bufs=3, space="PSUM") as sss, \
             tc.tile_pool(name="oq", bufs=1, space="PSUM") as oqp:
            cs_sb = consts.tile([P, s_len], F32)
            sc_sb = consts.tile([P, s_len], F32)
            t01_sb = consts.tile([P, P], BF16)
            idb_sb = consts.tile([P, P], BF16)
            ones_sb = consts.tile([P, nt], BF16)
            bias_sb = consts.tile([P, 6], F32)
            bias2_sb = consts.tile([P, 6], F32)

            qk_sb = qkpool.tile([P, 5 * s_len], BF16)   # q heads 0..3, k at 4
            vT_sb = vvpool.tile([P, s_len], BF16)       # [dk, s]
            v1_sb = vvpool.tile([P, nt * 129], BF16)    # [sk, dk|1] per tile
            ohn_sb = ohpool.tile([P, REP * nt * P], BF16)  # [sq, dk] normed
            w_sb = wpool.tile([P, 6 * D], BF16)
            woT_sb = wopool.tile([P, REP * D], BF16)

            xtiles = {}

            # ---------------- emission units ----------------
            def xdma(c):
                xq = xpool.tile([P, ND * CH], BF16, tag="x", name=f"xq{c}")
                xtiles[c] = xq
                for dt in range(ND):
                    nc.sync.dma_start(
                        out=xq[:, dt * CH:(dt + 1) * CH],
                        in_=xT[dt * P:(dt + 1) * P, c * CH:(c + 1) * CH])

            def mblock(c, m):
                """QKV projection m-block for chunk c (+RoPE / v-store)."""
                xq = xtiles[c]
                ps = pps.tile([P, CH], F32, tag="pp", name=f"ps{c}_{m}")
                for dt in range(ND):
                    nc.tensor.matmul(
                        ps, w_sb[:, m * D + dt * P: m * D + (dt + 1) * P],
                        xq[:, dt * CH:(dt + 1) * CH],
                        start=(dt == 0), stop=(dt == ND - 1))
                if m < 5:
                    # RoPE (partitions 0:64 real, 64:128 imag; see baseline)
                    cs_c = cs_sb[:, c * CH:(c + 1) * CH]
                    sc_c = sc_sb[:, c * CH:(c + 1) * CH]
                    add, mult = mybir.AluOpType.add, mybir.AluOpType.mult
                    u = rpool.tile([P, CH], F32, tag="p1")
                    v = rpool.tile([P, CH], F32, tag="p2")
                    nc.vector.scalar_tensor_tensor(
                        u[0:64], ps[0:64], bias_sb[0:64, m:m + 1],
                        cs_c[0:64], op0=add, op1=mult)
                    nc.vector.scalar_tensor_tensor(
                        u[64:128], ps[0:64], bias2_sb[64:128, m:m + 1],
                        cs_c[64:128], op0=add, op1=mult)
                    nc.vector.scalar_tensor_tensor(
                        v[0:64], ps[64:128], bias2_sb[0:64, m:m + 1],
                        sc_c[0:64], op0=add, op1=mult)
                    nc.vector.scalar_tensor_tensor(
                        v[64:128], ps[64:128], bias_sb[64:128, m:m + 1],
                        sc_c[64:128], op0=add, op1=mult)
                    dst = qk_sb[:, m * s_len + c * CH: m * s_len + (c + 1) * CH]
                    nc.vector.tensor_sub(dst[0:64], u[0:64], v[0:64])
                    nc.vector.tensor_add(dst[64:128], u[64:128], v[64:128])
                else:
                    nc.scalar.add(out=vT_sb[:, c * CH:(c + 1) * CH],
                                  in_=ps, add=bias_sb[:, m:m + 1])

            def vtrans(c, i):
                """Transpose v s-tile i of chunk c into v1 (+ones col)."""
                tt = c * (CH // P) + i
                tp = pps.tile([P, P], BF16, tag="pp", name=f"vt{tt}")
                nc.tensor.transpose(tp, vT_sb[:, tt * P:(tt + 1) * P], idb_sb)
                nc.any.tensor_copy(v1_sb[:, tt * 129: tt * 129 + 128], tp)
                nc.any.tensor_copy(v1_sb[:, tt * 129 + 128: tt * 129 + 129],
                                   ones_sb[:, tt:tt + 1])

            def oproj(st):
                """Output projection for sq-tile st (requires attn chunk
                st//4 finalized)."""
                ohts = otpool.tile([P, REP * P], BF16, tag="ohts",
                                   name=f"ohts{st}")
                for j in range(REP):
                    tp = pps.tile([P, P], BF16, tag="pp", name=f"ot{st}_{j}")
                    nc.tensor.transpose(
                        tp, ohn_sb[:, (j * nt + st) * P:(j * nt + st + 1) * P],
                        idb_sb)
                    nc.any.tensor_copy(ohts[:, j * P:(j + 1) * P], tp)
                fo = fopool.tile([P, D], F32, tag="fo", name=f"fo{st}")
                for dc in range(D // CH):
                    po = pps.tile([P, CH], F32, tag="pp", name=f"po{st}_{dc}")
                    for j in range(REP):
                        nc.tensor.matmul(
                            po, ohts[:, j * P:(j + 1) * P],
                            woT_sb[:, j * D + dc * CH: j * D + (dc + 1) * CH],
                            start=(j == 0), stop=(j == REP - 1))
                    nc.any.tensor_copy(fo[:, dc * CH:(dc + 1) * CH], po)
                    nc.sync.dma_start(
                        out=out[st * P:(st + 1) * P, dc * CH:(dc + 1) * CH],
                        in_=fo[:, dc * CH:(dc + 1) * CH])

            def wodma():
                for j in range(REP):
                    nc.scalar.dma_start(out=woT_sb[:, j * D:(j + 1) * D],
                                        in_=woT[j * P:(j + 1) * P, :])

            # ---------------- attention steps ----------------
            def oq_ap(oqt, lidx):
                bank, slot = divmod(lidx, 3)
                return oqt[bank][:, slot * 129: slot * 129 + 129]

            def emit_scores(c, p, t):
                es = []
                f0 = max(0, (t - 4 * c) * P)
                fn = CH - f0
                for hh in range(2):
                    h = 2 * p + hh
                    ss = sss.tile([P, CH], F32, tag="sc", name=f"ss{c}_{p}_{t}_{hh}")
                    nc.tensor.matmul(
                        ss[:, 0:fn],
                        qk_sb[:, 4 * s_len + t * P: 4 * s_len + (t + 1) * P],
                        qk_sb[:, h * s_len + c * CH + f0: h * s_len + (c + 1) * CH],
                        start=True, stop=True)
                    e = epool.tile([P, CH], BF16, tag=f"e{hh}", name=f"e{c}_{p}_{t}_{hh}")
                    nc.scalar.activation(e[:, 0:fn], ss[:, 0:fn], Exp,
                                         scale=scale)
                    if t >= 4 * c:
                        nc.vector.tensor_mul(e[:, 0:P], e[:, 0:P], t01_sb)
                    es.append((e, f0))
                return es

            def emit_av(c, p, t, es, oqt):
                # PSUM start_tensor_calc zeroes lazily at 2KB bank-region
                # granularity, so each bank gets exactly ONE start (its first
                # write, slot 0 at t=0) and ONE stop (its last write); the
                # other slots accumulate into the same hardware group and
                # read-as-zero on first touch. Bank layout (lidx = hh*4+sub,
                # slot = lidx%3): bank0 slots=subs(0,1,2) last@(lidx2,4c+2);
                # bank1 subs(3,0,1) last@(lidx3,4c+3); bank2 subs(2,3)
                # last@(lidx7,4c+3).
                bank_last = {0: (2, 4 * c + 2), 1: (3, 4 * c + 3),
                             2: (7, 4 * c + 3)}
                for hh in range(2):
                    e, f0 = es[hh]
                    for sub in range(max(0, t - 4 * c), 4):
                        lidx = hh * 4 + sub
                        bank, slot = divmod(lidx, 3)
                        ll, lt = bank_last[bank]
                        nc.tensor.matmul(
                            oq_ap(oqt, lidx),
                            e[:, sub * P - f0: (sub + 1) * P - f0],
                            v1_sb[:, t * 129: (t + 1) * 129],
                            start=(t == 0 and slot == 0),
                            stop=(lidx == ll and t == lt),
                            skip_group_check=False)

            def finalize_pass(c, p, oqt):
                rr = dpool.tile([P, 8], F32, tag="r", name=f"r{c}_{p}")
                for lidx in range(8):
                    nc.vector.reciprocal(rr[:, lidx:lidx + 1],
                                         oq_ap(oqt, lidx)[:, 128:129])
                for lidx in range(8):
                    hh, sub = divmod(lidx, 4)
                    h = 2 * p + hh
                    st = c * 4 + sub
                    nc.scalar.mul(
                        ohn_sb[:, (h * nt + st) * P:(h * nt + st + 1) * P],
                        oq_ap(oqt, lidx)[:, 0:128],
                        rr[:, lidx:lidx + 1])

            def attn_steps(c):
                """Yield per-step closures for chunk c (2 passes)."""
                for p in range(2):
                    oqt = [oqp.tile([P, 3 * 129], F32, tag=f"oq{b}",
                                    name=f"oq{c}_{p}_{b}") for b in range(3)]
                    state = {}

                    def first(p=p, oqt=oqt, state=state):
                        state["es"] = emit_scores(c, p, 0)
                    yield first
                    nsteps = 4 * c + 4
                    for t in range(1, nsteps):
                        def mid(t=t, p=p, oqt=oqt, state=state):
                            es_new = emit_scores(c, p, t)
                            emit_av(c, p, t - 1, state["es"], oqt)
                            state["es"] = es_new
                        yield mid

                    def last(p=p, oqt=oqt, state=state):
                        emit_av(c, p, nsteps - 1, state["es"], oqt)
                        finalize_pass(c, p, oqt)
                    yield last

            # ---------------- preamble ----------------
            nc.scalar.dma_start(out=bias_sb, in_=BIAS)
            nc.scalar.dma_start(out=bias2_sb, in_=BIAS2)
            nc.scalar.dma_start(out=w_sb[:, 0:D], in_=W[:, 0:D])
            xdma(0)
            nc.scalar.dma_start(out=cs_sb[:, 0:CH], in_=CSt[:, 0:CH])
            nc.scalar.dma_start(out=sc_sb[:, 0:CH], in_=SCt[:, 0:CH])
            for m in range(1, 6):
                nc.scalar.dma_start(out=w_sb[:, m * D:(m + 1) * D],
                                    in_=W[:, m * D:(m + 1) * D])
            nc.scalar.dma_start(out=t01_sb, in_=T01)
            nc.scalar.dma_start(out=idb_sb, in_=IDB)
            nc.scalar.dma_start(out=ones_sb, in_=ONESt)
            if s_len > CH:
                nc.scalar.dma_start(out=cs_sb[:, CH:s_len],
                                    in_=CSt[:, CH:s_len])
                nc.scalar.dma_start(out=sc_sb[:, CH:s_len],
                                    in_=SCt[:, CH:s_len])

            # chunk 0 QKV, unoverlapped (steady-state reps hide this)
            for m in range(6):
                mblock(0, m)
            for i in range(CH // P):
                vtrans(0, i)
            if nch > 1:
                xdma(1)
            else:
                wodma()

            # ---------------- windows ----------------
            for c in range(nch):
                fills = []
                if c + 1 < nch:
                    if c + 2 < nch:
                        fills.append(lambda cc=c + 2: xdma(cc))
                    if c == 0:
                        fills.append(wodma)
                    for m in range(6):
                        fills.append(lambda m=m, cc=c + 1: mblock(cc, m))
                    for i in range(CH // P):
                        fills.append(lambda i=i, cc=c + 1: vtrans(cc, i))
                else:
                    for st in range(max(0, nt - 4)):
                        fills.append(lambda st=st: oproj(st))

                steps = list(attn_steps(c))
                n, mfill = len(steps), len(fills)
                fi = 0
                for i, step in enumerate(steps):
                    while fi < mfill and fi * n <= i * mfill:
                        fills[fi]()
                        fi += 1
                    step()
                while fi < mfill:
                    fills[fi]()
                    fi += 1

            for st in range(max(0, nt - 4), nt):
                oproj(st)

    nc.compile()
    return nc


_PERM = np.concatenate([np.arange(0, DK, 2), np.arange(1, DK, 2)])


def _prep_device_inputs(x, freqs_cos, freqs_sin, wq_w, wq_b, wk_w, wk_b,
                        wv_w, wv_b, wo_w, s_len=S):
    f32 = np.float32

    def wtile(rows, permute):
        r = rows[_PERM] if permute else rows
        blk = np.ascontiguousarray(r.T).reshape(ND, P, P)
        return blk.transpose(1, 0, 2).reshape(P, D)

    cs = np.ascontiguousarray(
        np.concatenate([freqs_cos[:s_len].T, freqs_sin[:s_len].T], axis=0),
        dtype=f32)
    scm = np.ascontiguousarray(
        np.concatenate([freqs_sin[:s_len].T, freqs_cos[:s_len].T], axis=0),
        dtype=f32)
    pp, xx = np.meshgrid(np.arange(P), np.arange(P), indexing="ij")
    t01 = (xx >= pp).astype(NPBF)
    idb = np.eye(P, dtype=NPBF)
    ones = np.ones((P, s_len // P), dtype=NPBF)

    in_maps = []
    for d in range(NDEV):
        b, g = d // HK, d % HK
        xt = np.ascontiguousarray(x[b, :s_len].T).astype(NPBF)
        wblk = np.empty((P, 6 * D), dtype=f32)
        for m in range(REP):
            h = g * REP + m
            wblk[:, m * D:(m + 1) * D] = wtile(wq_w[h * P:(h + 1) * P], True)
        wblk[:, 4 * D:5 * D] = wtile(wk_w[g * P:(g + 1) * P], True)
        wblk[:, 5 * D:6 * D] = wtile(wv_w[g * P:(g + 1) * P], False)
        wot = np.concatenate(
            [np.ascontiguousarray(
                wo_w[:, (g * REP + j) * P:(g * REP + j + 1) * P].T)
             for j in range(REP)], axis=0).astype(NPBF)
        bias = np.zeros((P, 6), dtype=f32)
        for m in range(REP):
            h = g * REP + m
            bias[:, m] = wq_b[h * P:(h + 1) * P][_PERM]
        bias[:, 4] = wk_b[g * P:(g + 1) * P][_PERM]
        bias[:, 5] = wv_b[g * P:(g + 1) * P]
        in_maps.append({
            "xT": xt, "W": np.ascontiguousarray(wblk).astype(NPBF),
            "woT": wot, "CS": cs, "SC": scm, "T01": t01, "IDB": idb,
            "ONES": ones,
            "BIAS": np.ascontiguousarray(bias),
            "BIAS2": np.ascontiguousarray(np.roll(bias, 64, axis=0)),
        })
    return in_maps


_CACHE = {}


def _get_nc(s_len=S):
    if s_len not in _CACHE:
        _CACHE[s_len] = _build(s_len)
    return _CACHE[s_len]


def kernel(x, freqs_cos, freqs_sin, wq_w, wq_b, wk_w, wk_b, wv_w, wv_b,
           wo_w, wo_b, _trace=False):
    x = np.asarray(x, dtype=np.float32)
    args = [np.asarray(a, dtype=np.float32) for a in
            (freqs_cos, freqs_sin, wq_w, wq_b, wk_w, wk_b, wv_w, wv_b, wo_w)]
    wo_b = np.asarray(wo_b, dtype=np.float32)
    nc = _get_nc(S)
    in_maps = _prep_device_inputs(x, *args)
    res = run_bass_kernel_spmd(nc, in_maps, core_ids=list(range(NDEV)),
                               trace=_trace)
    outf = np.zeros((B, S, D), dtype=np.float32)
    for d in range(NDEV):
        outf[d // HK] += res.results[d]["out"]
    outf += wo_b[None, None, :]
    kernel.last_result = res
    return outf


# revision 3
# speedup vs baseline: 1.9109x; 1.9109x over previous
"""Trainium2 Bass kernel v2 for GQA attention (B=2, S=2048, D=2048, H=16, HK=4).

Sharding: 8 devices = batch(2) x kv-groups(4), as baseline: each device owns
one batch element and one GQA group (4 q-heads + 1 kv-head); wq/wk/wv
column-parallel, wo row-parallel (host sums the 4 partials per batch element).

v2 changes vs baseline:
  - all matmuls bf16 (1 cycle/row at ANY moving width; fp32r needed >=256).
    Numerics verified host-side: all-bf16 absmax-rel err 3.1e-3 << 2e-2 gate.
  - denominator row-sum matmuls (dd, ~72K PE rows) and one-hot broadcast
    matmuls (bb, 8K rows) eliminated: AV is computed in [sq, dk] orientation
    (exp tile stationary, v moving) with a ones-column appended to v, so the
    softmax denominator rides along as output column 128. Normalization is
    then a per-partition scalar multiply on ACT (native broadcast).
  - causal diagonal handled by a multiplicative 0/1 triangle on the exp tile
    (DVE, bf16 2x) instead of additive -1e9 + padded fp32r tiles; score
    tiles use exact widths {512,384,256,128}.
  - oh tiles transposed back to [dk, sq] on PE (bf16, 64x128 rows) for the
    row-parallel wo matmul.
  - QKV projection (chunk c+1) and O-proj (ready sq-tiles) are interleaved
    into the attention t-loop so the ACT exp stream (~100us) hides under
    projection PE work instead of stalling the attention phase.

PSUM budget: pp(2: QKV out + transposes + O-proj out) + ss(3: scores) +
oq(3: 8 attention accumulators packed 3x129 cols per bank) = 8 banks.
"""

import math

import numpy as np
import ml_dtypes

import concourse.bacc as bacc
import concourse.tile as tile
from concourse import mybir
from concourse.bass_utils import run_bass_kernel_spmd

B, S, D = 2, 2048, 2048
H, HK, DK = 16, 4, 128
REP = H // HK
NDEV = 8
P = 128
CH = 512
ND = D // P          # 16 k-step tiles per matmul
NT = S // P          # 16 sk/sq tiles
F32 = mybir.dt.float32
BF16 = mybir.dt.bfloat16
NPBF = ml_dtypes.bfloat16


def _build(s_len=S, reps=1):
    nch = s_len // CH
    nt = s_len // P
    scale = 1.0 / math.sqrt(DK)
    Exp = mybir.ActivationFunctionType.Exp

    nc = bacc.Bacc("TRN2", target_bir_lowering=False, debug=False,
                   enable_asserts=False, num_devices=1)
    xT = nc.dram_tensor("xT", [D, s_len], BF16, kind="ExternalInput").ap()
    W = nc.dram_tensor("W", [P, 6 * D], BF16, kind="ExternalInput").ap()
    woT = nc.dram_tensor("woT", [REP * DK, D], BF16, kind="ExternalInput").ap()
    CSt = nc.dram_tensor("CS", [P, s_len], F32, kind="ExternalInput").ap()
    SCt = nc.dram_tensor("SC", [P, s_len], F32, kind="ExternalInput").ap()
    T01 = nc.dram_tensor("T01", [P, P], BF16, kind="ExternalInput").ap()
    IDB = nc.dram_tensor("IDB", [P, P], BF16, kind="ExternalInput").ap()
    ONESt = nc.dram_tensor("ONES", [P, nt], BF16, kind="ExternalInput").ap()
    BIAS = nc.dram_tensor("BIAS", [P, 6], F32, kind="ExternalInput").ap()
    BIAS2 = nc.dram_tensor("BIAS2", [P, 6], F32, kind="ExternalInput").ap()
    out = nc.dram_tensor("out", [s_len, D], BF16, kind="ExternalOutput").ap()

    with tile.TileContext(nc) as tc:
      for _rep in range(reps):
        with tc.tile_pool(name="consts", bufs=1) as consts, \
             tc.tile_pool(name="qk", bufs=1) as qkpool, \
             tc.tile_pool(name="vv", bufs=1) as vvpool, \
             tc.tile_pool(name="oh", bufs=1) as ohpool, \
             tc.tile_pool(name="wst", bufs=1) as wpool, \
             tc.tile_pool(name="wo", bufs=1) as wopool, \
             tc.tile_pool(name="xh", bufs=2) as xpool, \
             tc.tile_pool(name="rope", bufs=3) as rpool, \
             tc.tile_pool(name="ew", bufs=2) as epool, \
             tc.tile_pool(name="oht", bufs=2) as otpool, \
             tc.tile_pool(name="fo", bufs=3) as fopool, \
             tc.tile_pool(name="den", bufs=2) as dpool, \
             tc.tile_pool(name="pp", bufs=2, space="PSUM") as pps, \
             tc.tile_pool(name="ss", bufs=3, space="PSUM") as sss, \
             tc.tile_pool(name="oq", bufs=1, space="PSUM") as oqp:
            cs_sb = consts.tile([P, s_len], F32)
            sc_sb = consts.tile([P, s_len], F32)
            t01_sb = consts.tile([P, P], BF16)
            idb_sb = consts.tile([P, P], BF16)
            ones_sb = consts.tile([P, nt], BF16)
            bias_sb = consts.tile([P, 6], F32)
            bias2_sb = consts.tile([P, 6], F32)

            qk_sb = qkpool.tile([P, 5 * s_len], BF16)   # q heads 0..3, k at 4
            vT_sb = vvpool.tile([P, s_len], BF16)       # [dk, s]
            v1_sb = vvpool.tile([P, nt * 129], BF16)    # [sk, dk|1] per tile
            ohn_sb = ohpool.tile([P, REP * nt * P], BF16)  # [sq, dk] normed
            w_sb = wpool.tile([P, 6 * D], BF16)
            woT_sb = wopool.tile([P, REP * D], BF16)

            xtiles = {}

            # ---------------- emission units ----------------
            def xdma(c):
                xq = xpool.tile([P, ND * CH], BF16, tag="x", name=f"xq{c}")
                xtiles[c] = xq
                for dt in range(ND):
                    nc.sync.dma_start(
                        out=xq[:, dt * CH:(dt + 1) * CH],
                        in_=xT[dt * P:(dt + 1) * P, c * CH:(c + 1) * CH])

            def mblock(c, m):
                """QKV projection m-block for chunk c (+RoPE / v-store)."""
                xq = xtiles[c]
                ps = pps.tile([P, CH], F32, tag="pp", name=f"ps{c}_{m}")
                for dt in range(ND):
                    nc.tensor.matmul(
                        ps, w_sb[:, m * D + dt * P: m * D + (dt + 1) * P],
                        xq[:, dt * CH:(dt + 1) * CH],
                        start=(dt == 0), stop=(dt == ND - 1))
                if m < 5:
                    # RoPE (partitions 0:64 real, 64:128 imag; see baseline)
                    cs_c = cs_sb[:, c * CH:(c + 1) * CH]
                    sc_c = sc_sb[:, c * CH:(c + 1) * CH]
                    add, mult = mybir.AluOpType.add, mybir.AluOpType.mult
                    u = rpool.tile([P, CH], F32, tag="p1")
                    v = rpool.tile([P, CH], F32, tag="p2")
                    nc.vector.scalar_tensor_tensor(
                        u[0:64], ps[0:64], bias_sb[0:64, m:m + 1],
                        cs_c[0:64], op0=add, op1=mult)
                    nc.vector.scalar_tensor_tensor(
                        u[64:128], ps[0:64], bias2_sb[64:128, m:m + 1],
                        cs_c[64:128], op0=add, op1=mult)
                    nc.vector.scalar_tensor_tensor(
                        v[0:64], ps[64:128], bias2_sb[0:64, m:m + 1],
                        sc_c[0:64], op0=add, op1=mult)
                    nc.vector.scalar_tensor_tensor(
                        v[64:128], ps[64:128], bias_sb[64:128, m:m + 1],
                        sc_c[64:128], op0=add, op1=mult)
                    dst = qk_sb[:, m * s_len + c * CH: m * s_len + (c + 1) * CH]
                    nc.vector.tensor_sub(dst[0:64], u[0:64], v[0:64])
                    nc.vector.tensor_add(dst[64:128], u[64:128], v[64:128])
                else:
                    nc.scalar.add(out=vT_sb[:, c * CH:(c + 1) * CH],
                                  in_=ps, add=bias_sb[:, m:m + 1])

            def vtrans(c, i):
                """Transpose v s-tile i of chunk c into v1 (+ones col)."""
                tt = c * (CH // P) + i
                tp = pps.tile([P, P], BF16, tag="pp", name=f"vt{tt}")
                nc.tensor.transpose(tp, vT_sb[:, tt * P:(tt + 1) * P], idb_sb)
                nc.any.tensor_copy(v1_sb[:, tt * 129: tt * 129 + 128], tp)
                nc.any.tensor_copy(v1_sb[:, tt * 129 + 128: tt * 129 + 129],
                                   ones_sb[:, tt:tt + 1])

            def oproj(st):
                """Output projection for sq-tile st (requires attn chunk
                st//4 finalized)."""
                ohts = otpool.tile([P, REP * P], BF16, tag="ohts",
                                   name=f"ohts{st}")
                for j in range(REP):
                    tp = pps.tile([P, P], BF16, tag="pp", name=f"ot{st}_{j}")
                    nc.tensor.transpose(
                        tp, ohn_sb[:, (j * nt + st) * P:(j * nt + st + 1) * P],
                        idb_sb)
                    nc.any.tensor_copy(ohts[:, j * P:(j + 1) * P], tp)
                fo = fopool.tile([P, D], BF16, tag="fo", name=f"fo{st}")
                for dc in range(D // CH):
                    po = pps.tile([P, CH], F32, tag="pp", name=f"po{st}_{dc}")
                    for j in range(REP):
                        nc.tensor.matmul(
                            po, ohts[:, j * P:(j + 1) * P],
                            woT_sb[:, j * D + dc * CH: j * D + (dc + 1) * CH],
                            start=(j == 0), stop=(j == REP - 1))
                    nc.any.tensor_copy(fo[:, dc * CH:(dc + 1) * CH], po)
                    nc.sync.dma_start(
                        out=out[st * P:(st + 1) * P, dc * CH:(dc + 1) * CH],
                        in_=fo[:, dc * CH:(dc + 1) * CH])

            def wodma():
                for j in range(REP):
                    nc.scalar.dma_start(out=woT_sb[:, j * D:(j + 1) * D],
                                        in_=woT[j * P:(j + 1) * P, :])

            # ---------------- attention steps ----------------
            def oq_ap(oqt, lidx):
                bank, slot = divmod(lidx, 3)
                return oqt[bank][:, slot * 129: slot * 129 + 129]

            def emit_scores(c, p, t):
                es = []
                f0 = max(0, (t - 4 * c) * P)
                fn = CH - f0
                for hh in range(2):
                    h = 2 * p + hh
                    ss = sss.tile([P, CH], F32, tag="sc", name=f"ss{c}_{p}_{t}_{hh}")
                    nc.tensor.matmul(
                        ss[:, 0:fn],
                        qk_sb[:, 4 * s_len + t * P: 4 * s_len + (t + 1) * P],
                        qk_sb[:, h * s_len + c * CH + f0: h * s_len + (c + 1) * CH],
                        start=True, stop=True)
                    e = epool.tile([P, CH], BF16, tag=f"e{hh}", name=f"e{c}_{p}_{t}_{hh}")
                    nc.scalar.activation(e[:, 0:fn], ss[:, 0:fn], Exp,
                                         scale=scale)
                    if t >= 4 * c:
                        nc.vector.tensor_mul(e[:, 0:P], e[:, 0:P], t01_sb)
                    es.append((e, f0))
                return es

            def emit_av(c, p, t, es, oqt):
                # PSUM start_tensor_calc zeroes lazily at 2KB bank-region
                # granularity, so each bank gets exactly ONE start (its first
                # write, slot 0 at t=0) and ONE stop (its last write); the
                # other slots accumulate into the same hardware group and
                # read-as-zero on first touch. Bank layout (lidx = hh*4+sub,
                # slot = lidx%3): bank0 slots=subs(0,1,2) last@(lidx2,4c+2);
                # bank1 subs(3,0,1) last@(lidx3,4c+3); bank2 subs(2,3)
                # last@(lidx7,4c+3).
                bank_last = {0: (2, 4 * c + 2), 1: (3, 4 * c + 3),
                             2: (7, 4 * c + 3)}
                for hh in range(2):
                    e, f0 = es[hh]
                    for sub in range(max(0, t - 4 * c), 4):
                        lidx = hh * 4 + sub
                        bank, slot = divmod(lidx, 3)
                        ll, lt = bank_last[bank]
                        nc.tensor.matmul(
                            oq_ap(oqt, lidx),
                            e[:, sub * P - f0: (sub + 1) * P - f0],
                            v1_sb[:, t * 129: (t + 1) * 129],
                            start=(t == 0 and slot == 0),
                            stop=(lidx == ll and t == lt),
                            skip_group_check=False)

            def finalize_pass(c, p, oqt):
                rr = dpool.tile([P, 8], F32, tag="r", name=f"r{c}_{p}")
                for lidx in range(8):
                    nc.vector.reciprocal(rr[:, lidx:lidx + 1],
                                         oq_ap(oqt, lidx)[:, 128:129])
                for lidx in range(8):
                    hh, sub = divmod(lidx, 4)
                    h = 2 * p + hh
                    st = c * 4 + sub
                    nc.scalar.mul(
                        ohn_sb[:, (h * nt + st) * P:(h * nt + st + 1) * P],
                        oq_ap(oqt, lidx)[:, 0:128],
                        rr[:, lidx:lidx + 1])

            def attn_steps(c):
                """Yield per-step closures for chunk c (2 passes)."""
                for p in range(2):
                    oqt = [oqp.tile([P, 3 * 129], F32, tag=f"oq{b}",
                                    name=f"oq{c}_{p}_{b}") for b in range(3)]
                    state = {}

                    def first(p=p, oqt=oqt, state=state):
                        state["es"] = emit_scores(c, p, 0)
                    yield first
                    nsteps = 4 * c + 4
                    for t in range(1, nsteps):
                        def mid(t=t, p=p, oqt=oqt, state=state):
                            es_new = emit_scores(c, p, t)
                            emit_av(c, p, t - 1, state["es"], oqt)
                            state["es"] = es_new
                        yield mid

                    def last(p=p, oqt=oqt, state=state):
                        emit_av(c, p, nsteps - 1, state["es"], oqt)
                        finalize_pass(c, p, oqt)
                    yield last

            # ---------------- preamble ----------------
            nc.scalar.dma_start(out=bias_sb, in_=BIAS)
            nc.scalar.dma_start(out=bias2_sb, in_=BIAS2)
            nc.scalar.dma_start(out=w_sb[:, 0:D], in_=W[:, 0:D])
            xdma(0)
            nc.scalar.dma_start(out=cs_sb[:, 0:CH], in_=CSt[:, 0:CH])
            nc.scalar.dma_start(out=sc_sb[:, 0:CH], in_=SCt[:, 0:CH])
            for m in range(1, 6):
                nc.scalar.dma_start(out=w_sb[:, m * D:(m + 1) * D],
                                    in_=W[:, m * D:(m + 1) * D])
            nc.scalar.dma_start(out=t01_sb, in_=T01)
            nc.scalar.dma_start(out=idb_sb, in_=IDB)
            nc.scalar.dma_start(out=ones_sb, in_=ONESt)
            if s_len > CH:
                nc.scalar.dma_start(out=cs_sb[:, CH:s_len],
                                    in_=CSt[:, CH:s_len])
                nc.scalar.dma_start(out=sc_sb[:, CH:s_len],
                                    in_=SCt[:, CH:s_len])

            # chunk 0 QKV, unoverlapped (steady-state reps hide this)
            for m in range(6):
                mblock(0, m)
            for i in range(CH // P):
                vtrans(0, i)
            if nch > 1:
                xdma(1)
            else:
                wodma()

            # ---------------- windows ----------------
            for c in range(nch):
                fills = []
                if c + 1 < nch:
                    if c + 2 < nch:
                        fills.append(lambda cc=c + 2: xdma(cc))
                    if c == 0:
                        fills.append(wodma)
                    for m in range(6):
                        fills.append(lambda m=m, cc=c + 1: mblock(cc, m))
                    for i in range(CH // P):
                        fills.append(lambda i=i, cc=c + 1: vtrans(cc, i))
                else:
                    for st in range(max(0, nt - 4)):
                        fills.append(lambda st=st: oproj(st))

                steps = list(attn_steps(c))
                n, mfill = len(steps), len(fills)
                fi = 0
                for i, step in enumerate(steps):
                    while fi < mfill and fi * n <= i * mfill:
                        fills[fi]()
                        fi += 1
                    step()
                while fi < mfill:
                    fills[fi]()
                    fi += 1

            for st in range(max(0, nt - 4), nt):
                oproj(st)

    nc.compile()
    return nc


_PERM = np.concatenate([np.arange(0, DK, 2), np.arange(1, DK, 2)])


def _prep_device_inputs(x, freqs_cos, freqs_sin, wq_w, wq_b, wk_w, wk_b,
                        wv_w, wv_b, wo_w, s_len=S):
    f32 = np.float32

    def wtile(rows, permute):
        r = rows[_PERM] if permute else rows
        blk = np.ascontiguousarray(r.T).reshape(ND, P, P)
        return blk.transpose(1, 0, 2).reshape(P, D)

    cs = np.ascontiguousarray(
        np.concatenate([freqs_cos[:s_len].T, freqs_sin[:s_len].T], axis=0),
        dtype=f32)
    scm = np.ascontiguousarray(
        np.concatenate([freqs_sin[:s_len].T, freqs_cos[:s_len].T], axis=0),
        dtype=f32)
    pp, xx = np.meshgrid(np.arange(P), np.arange(P), indexing="ij")
    t01 = (xx >= pp).astype(NPBF)
    idb = np.eye(P, dtype=NPBF)
    ones = np.ones((P, s_len // P), dtype=NPBF)

    in_maps = []
    for d in range(NDEV):
        b, g = d // HK, d % HK
        xt = np.ascontiguousarray(x[b, :s_len].T).astype(NPBF)
        wblk = np.empty((P, 6 * D), dtype=f32)
        for m in range(REP):
            h = g * REP + m
            wblk[:, m * D:(m + 1) * D] = wtile(wq_w[h * P:(h + 1) * P], True)
        wblk[:, 4 * D:5 * D] = wtile(wk_w[g * P:(g + 1) * P], True)
        wblk[:, 5 * D:6 * D] = wtile(wv_w[g * P:(g + 1) * P], False)
        wot = np.concatenate(
            [np.ascontiguousarray(
                wo_w[:, (g * REP + j) * P:(g * REP + j + 1) * P].T)
             for j in range(REP)], axis=0).astype(NPBF)
        bias = np.zeros((P, 6), dtype=f32)
        for m in range(REP):
            h = g * REP + m
            bias[:, m] = wq_b[h * P:(h + 1) * P][_PERM]
        bias[:, 4] = wk_b[g * P:(g + 1) * P][_PERM]
        bias[:, 5] = wv_b[g * P:(g + 1) * P]
        in_maps.append({
            "xT": xt, "W": np.ascontiguousarray(wblk).astype(NPBF),
            "woT": wot, "CS": cs, "SC": scm, "T01": t01, "IDB": idb,
            "ONES": ones,
            "BIAS": np.ascontiguousarray(bias),
            "BIAS2": np.ascontiguousarray(np.roll(bias, 64, axis=0)),
        })
    return in_maps


_CACHE = {}


def _get_nc(s_len=S):
    if s_len not in _CACHE:
        _CACHE[s_len] = _build(s_len)
    return _CACHE[s_len]


def kernel(x, freqs_cos, freqs_sin, wq_w, wq_b, wk_w, wk_b, wv_w, wv_b,
           wo_w, wo_b, _trace=False):
    x = np.asarray(x, dtype=np.float32)
    args = [np.asarray(a, dtype=np.float32) for a in
            (freqs_cos, freqs_sin, wq_w, wq_b, wk_w, wk_b, wv_w, wv_b, wo_w)]
    wo_b = np.asarray(wo_b, dtype=np.float32)
    nc = _get_nc(S)
    in_maps = _prep_device_inputs(x, *args)
    res = run_bass_kernel_spmd(nc, in_maps, core_ids=list(range(NDEV)),
                               trace=_trace)
    outf = np.zeros((B, S, D), dtype=np.float32)
    for d in range(NDEV):
        outf[d // HK] += res.results[d]["out"].astype(np.float32)
    outf += wo_b[None, None, :]
    kernel.last_result = res
    return outf
